# revision 43
# baseline (speedup 1.0000x reference)
"""MixedQLinear (QUIK-style int4+fp16 outlier linear) on 8 TRN2 NeuronCores.

Sharding: token-parallel. x [4,2048,4096] -> 8192 tokens, 1024 per core;
weights replicated. Each core quantizes its tokens, runs the int4 GEMM in
fp8e4 DoubleRow perf mode (exact: int4 values/products are exactly
representable in e4m3 and accumulate exactly in fp32 PSUM), plus the
fp16 outlier GEMM, dequantizes, and writes its [1024,4096] output slice.

Key algebra: with r = clip(round((x-mn)/scale),0,15)  (no -8 shift),
  out = (sum_k r*Wint) * scale * ws  +  mn*reduced_w  +  fp_x@Wfp^T + bias
(the -8 shift of the reference cancels exactly against zero*reduced_w).
mn (an exact f16 value: the min of f16 inputs) and a ones-row ride as two
extra contraction rows of the fp-outlier matmul with weights [bias; rw].

Quantize trick: r+1024 = x*inv + (1024 - mn*inv) computed on the scalar
engine in one activation; f16 spacing on [1024,2048) is exactly 1.0, so the
f32->f16 output conversion IS round-to-nearest-even onto the integer grid.
The -1024 is subtracted in the PSUM->SBUF fp8 conversion after the PE
transpose (exact integers throughout).

Pipeline: per 128-token tile: load -> min/max -> quantize -> transpose ->
fp8; the tile's n=0 matmuls are emitted immediately after, so the tensor
engine starts ~10us in instead of waiting ~250us for full quantization
(the baseline's big stall).
"""

import numpy as np
import ml_dtypes
import concourse.bass as bass
import concourse.tile as tile
import concourse.mybir as mybir
from concourse.bass_utils import run_bass_kernel_spmd
from concourse.masks import make_identity
from bass_rust import ScopedClock, SyncInfo
from concourse.alu_op_type import AluOpType

# ---------------------------------------------------------------------------
# Workaround: this toolchain's walrus accepts at most one sync-wait on a
# TPB_CTRL (Drain) instruction; Tile's tail drain attaches one wait per
# active DMA queue. Split it into a chain of single-wait drains.
def _drain_and_barrier(self, tick_clock, wait_clock):
    drain_inst = self.nc.sync.drain()
    wait_clock.add_sem_waits(
        drain_inst.ins, ScopedClock({None: tick_clock.global_clock})
    )
    si = drain_inst.ins.sync_info
    ow = list(si.on_wait) if si is not None else []
    if len(ow) > 1:
        si.on_wait = [ow[0]]
        for w in ow[1:]:
            d2 = self.nc.sync.drain()
            d2.ins.sync_info = SyncInfo(on_wait=[w], on_update=[])
    self.nc.all_engine_barrier()
    assert self.sems is not None
    popped = self.nc._tile_sem_poison_stack.pop()
    assert popped is self._sem_poison
    self.nc.clear_and_free_semaphores(list(self.sems.allocated().values()))
    self.nc.all_engine_barrier()


tile.TileContext._drain_and_barrier = _drain_and_barrier


def _split_multiwait_instructions(nc):
    """Walrus here allows only one sync-wait per instruction: hoist extra
    waits onto same-engine NOPs inserted immediately before."""
    ctr = 0
    for fn in nc.m.functions:
        for bb in fn.blocks:
            insts = bb.instructions
            out = []
            changed = False
            for ins in insts:
                si = getattr(ins, "sync_info", None)
                ow = list(si.on_wait) if si is not None else []
                if len(ow) > 1:
                    changed = True
                    for w in ow[:-1]:
                        ctr += 1
                        out.append(
                            mybir.InstNoOp(
                                name=f"mwsplit-{ctr}",
                                sync_info=SyncInfo(on_wait=[w], on_update=[]),
                                engine=ins.engine,
                                bass_nofuse=True,
                            )
                        )
                    si.on_wait = [ow[-1]]
                out.append(ins)
            if changed:
                bb.instructions = out
# ---------------------------------------------------------------------------

N_CORES = 8
B, S, IN, OUT, FP = 4, 2048, 4096, 4096, 256
INT = IN - FP                    # 3840 int-quantized features
NT = (B * S) // N_CORES          # 1024 tokens per core
P = 128
KC = INT // P                    # 30 feature chunks
KH = KC // 2                     # 15 chunks per transpose-staging half
NOUT = 8                         # out-feature chunks
NSZ = OUT // NOUT                # 512
TOKT = NT // P                   # 8 token tiles of 128
BIG = 30000.0

f16 = mybir.dt.float16
f32 = mybir.dt.float32
f8 = mybir.dt.float8e4

_prog_cache = {}


def _build_program():
    nc = bass.Bass()
    xs = nc.declare_dram_parameter("xs", [NT, INT], f16, isOutput=False)
    fpx = nc.declare_dram_parameter("fpx", [NT, FP], f16, isOutput=False)
    wq = nc.declare_dram_parameter("wq", [NOUT, P, KC, NSZ], f8, isOutput=False)
    wfp0 = nc.declare_dram_parameter("wfp0", [P, OUT], f16, isOutput=False)
    wfp1 = nc.declare_dram_parameter("wfp1", [P, OUT], f16, isOutput=False)
    wsrow = nc.declare_dram_parameter("wsrow", [OUT], f16, isOutput=False)
    rwrow = nc.declare_dram_parameter("rwrow", [OUT], f16, isOutput=False)
    out_d = nc.declare_dram_parameter("out", [NT, OUT], f16, isOutput=True)

    def bcast(ap, parts=P):
        # DRAM row -> all partitions: stride-0 partition dim, SWDGE DMA
        return bass.AP(
            tensor=ap.tensor, offset=ap.offset, ap=[[0, parts]] + list(ap.ap)
        )

    HSZ = NT // 2
    DR = mybir.MatmulPerfMode.DoubleRow

    with tile.TileContext(nc) as tc:
        with (
            tc.tile_pool(name="const", bufs=1) as cpool,
            tc.tile_pool(name="wqp", bufs=3) as wqpool,
            tc.tile_pool(name="xp", bufs=1) as xpool,
            tc.tile_pool(name="r16p", bufs=1) as r16pool,
            tc.tile_pool(name="rtp", bufs=1) as rtpool,
            tc.tile_pool(name="s1", bufs=1) as s1pool,
            tc.tile_pool(name="scr", bufs=1) as scrpool,
            tc.tile_pool(name="dq", bufs=2) as dqpool,
            tc.tile_pool(name="pi", bufs=2, space="PSUM") as pipool,
            tc.tile_pool(name="pf", bufs=2, space="PSUM") as pfpool,
            tc.tile_pool(name="st", bufs=2, space="PSUM") as stpool,
            tc.tile_pool(name="dram", bufs=1, space="DRAM") as dpool,
        ):
            # ---- resident constants -------------------------------------
            # identity is built first: every PE transpose waits on it, and the
            # gpsimd queue is in-order
            ident = cpool.tile([P, P], f16, tag="ident")
            make_identity(nc, ident[:])

            # fp-outlier activations: natural (fast) DMA first on the sync
            # ring, then transposed on the PE -- the scheduler puts the (0,0)
            # fp matmuls at the head of the in-order PE queue, so fpt must be
            # ready early, and transposed DMAs are far too slow for that
            # x(0), x(1) lead the sync ring (they gate the quantize ramp),
            # then fpx (needed by ~25us), then the remaining x tiles;
            # weights ride the scalar ring so neither blocks the other
            xts = [None] * TOKT
            for t in range(2):
                ts_ = slice(t * P, (t + 1) * P)
                xt = xpool.tile([P, INT], f16, name=f"xt{t}", tag=f"xt{t}")
                nc.sync.dma_start(xt[:], xs[ts_, :])
                xts[t] = xt
            fxts = []
            for t in range(TOKT):
                ts_ = slice(t * P, (t + 1) * P)
                fxt = cpool.tile([P, FP], f16, name=f"fxt{t}", tag=f"fxt{t}")
                nc.sync.dma_start(fxt[:], fpx[ts_, :])
                fxts.append(fxt)
            for t in range(2, TOKT):
                ts_ = slice(t * P, (t + 1) * P)
                # last two tiles reuse the first two buffers (WAR-sequenced
                # behind their consumption, still ahead of the pipeline)
                xt = xpool.tile([P, INT], f16, name=f"xt{t}", tag=f"xt{t % 6}")
                nc.sync.dma_start(xt[:], xs[ts_, :])
                xts[t] = xt
            # ws/rw rows first on the scalar ring (tiny, needed early by the
            # PE outer-product broadcasts), then fp weights, then wq chunks
            wsrow_s = cpool.tile([1, OUT], f16, tag="wsrow_s")
            nc.scalar.dma_start(wsrow_s[:], wsrow[:].rearrange("(a j) -> a j", a=1))
            rwrow_s = cpool.tile([1, OUT], f16, tag="rwrow_s")
            nc.scalar.dma_start(rwrow_s[:], rwrow[:].rearrange("(a j) -> a j", a=1))
            wfp0_s = cpool.tile([P, OUT], f16, tag="wfp0")
            nc.scalar.dma_start(wfp0_s[:], wfp0[:])
            wfp1_s = cpool.tile([P, OUT], f16, tag="wfp1")
            nc.scalar.dma_start(wfp1_s[:], wfp1[:])
            # wsB/rwB = ones x row outer products on the PE (SWDGE bcast is slow)
            ones1 = cpool.tile([1, P], f16, tag="ones1")
            nc.vector.memset(ones1[:], 1.0)
            wsB = cpool.tile([P, OUT], f16, tag="wsB")
            rwB = cpool.tile([P, OUT], f16, tag="rwB")
            for n in range(NOUT):
                ns = slice(n * NSZ, (n + 1) * NSZ)
                pws = pfpool.tile([P, NSZ], f32, name="pws", tag="pf")
                nc.tensor.matmul(pws[:], ones1[:], wsrow_s[:, ns],
                                 start=True, stop=True)
                nc.scalar.copy(wsB[:, ns], pws[:])
                prw = pfpool.tile([P, NSZ], f32, name="prw", tag="pf")
                nc.tensor.matmul(prw[:], ones1[:], rwrow_s[:, ns],
                                 start=True, stop=True)
                nc.scalar.copy(rwB[:, ns], prw[:])
            # PE-transpose fpx tiles into [feat, tok] layout
            fpt0 = cpool.tile([P, NT], f16, tag="fpt0")
            fpt1 = cpool.tile([P, NT], f16, tag="fpt1")
            for t in range(TOKT):
                ts_ = slice(t * P, (t + 1) * P)
                stgf = stpool.tile([P, KH, P], f16, name="stgf", tag="stg")
                nc.tensor.transpose(stgf[:, 0, :], fxts[t][:, 0:P], ident[:])
                nc.tensor.transpose(stgf[:, 1, :], fxts[t][:, P:FP], ident[:])
                nc.scalar.copy(fpt0[:, ts_], stgf[:, 0, :])
                nc.scalar.copy(fpt1[:, ts_], stgf[:, 1, :])

            def load_wq(n):
                w = wqpool.tile([P, KC, NSZ], f8, tag="wq")
                nc.sync.dma_start(w[:], wq[n])
                return w

            scl = [None] * TOKT   # per-tile [128,1] f32 scale (dequant)
            zro = [None] * TOKT   # per-tile [128,1] f32 zero point
            rts = [None] * TOKT   # per-tile [128, KC, 128] fp8 r values

            def quantize_tile(t):
                ts_ = slice(t * P, (t + 1) * P)
                xt = xts[t]
                mx_t = s1pool.tile([P, 1], f32, tag=f"mx{t}")
                mn_t = s1pool.tile([P, 1], f32, tag=f"mn{t}")
                scr = scrpool.tile([P, INT // 2], f16, tag="scr")
                nc.vector.tensor_tensor(
                    scr[:], xt[:, : INT // 2], xt[:, INT // 2 :], AluOpType.max
                )
                nc.vector.tensor_reduce(
                    mx_t[:], scr[:], mybir.AxisListType.X, AluOpType.max
                )
                scr2 = scrpool.tile([P, INT // 2], f16, tag="scr")
                nc.vector.tensor_tensor(
                    scr2[:], xt[:, : INT // 2], xt[:, INT // 2 :], AluOpType.min
                )
                nc.vector.tensor_reduce(
                    mn_t[:], scr2[:], mybir.AxisListType.X, AluOpType.min
                )
                sc_t = s1pool.tile([P, 1], f32, tag=f"sc{t}")
                nc.vector.tensor_tensor(sc_t[:], mx_t[:], mn_t[:], AluOpType.subtract)
                nc.vector.tensor_scalar(
                    sc_t[:], sc_t[:], 1.0 / 15.0, 1e-8,
                    AluOpType.mult, AluOpType.max,
                )
                inv_t = s1pool.tile([P, 1], f32, tag=f"inv{t}")
                nc.vector.reciprocal(inv_t[:], sc_t[:])
                # nmi = 1024 - mn*inv (quantize bias)
                nmi = s1pool.tile([P, 1], f32, tag=f"nmi{t}")
                nc.vector.tensor_tensor(nmi[:], mn_t[:], inv_t[:], AluOpType.mult)
                nc.vector.tensor_scalar(
                    nmi[:], nmi[:], -1.0, 1024.0, AluOpType.mult, AluOpType.add
                )
                # zero point zero = 8*scale + mn (stays f32 for dequant)
                zr = s1pool.tile([P, 1], f32, tag=f"zr{t}")
                nc.vector.tensor_scalar(
                    zr[:], sc_t[:], 8.0, mn_t[:, 0:1], AluOpType.mult, AluOpType.add
                )
                # r+1024 in one Pool op; f16 write rounds to the int grid
                r16 = r16pool.tile([P, INT], f16, tag="r16")
                nc.gpsimd.tensor_scalar(
                    r16[:], xt[:], inv_t[:, 0:1], nmi[:, 0:1],
                    AluOpType.mult, AluOpType.add,
                )
                rt = rtpool.tile([P, KC, P], f8, tag=f"rt{t}")
                for h in range(2):
                    stg = stpool.tile([P, KH, P], f16, tag="stg")
                    for kk in range(KH):
                        c = h * KH + kk
                        nc.tensor.transpose(
                            stg[:, kk, :], r16[:, c * P : (c + 1) * P], ident[:]
                        )
                    nc.scalar.activation(
                        rt[:, h * KH : (h + 1) * KH, :], stg[:],
                        mybir.ActivationFunctionType.Copy, bias=-1032.0,
                    )
                scl[t] = sc_t
                zro[t] = zr
                rts[t] = rt

            def mm_tile(n, t, w, dve_mult=False):
                ts_ = slice(t * P, (t + 1) * P)
                ns = slice(n * NSZ, (n + 1) * NSZ)
                psum_i = pipool.tile([P, NSZ], f32, tag="pi")
                for kp in range(KC // 2):
                    nc.tensor.matmul(
                        psum_i[:],
                        rts[t][:, 2 * kp : 2 * kp + 2, :],
                        w[:, 2 * kp : 2 * kp + 2, :],
                        start=(kp == 0), stop=(kp == KC // 2 - 1),
                        perf_mode=DR,
                    )
                psum_f = pfpool.tile([P, NSZ], f32, tag="pf")
                nc.tensor.matmul(
                    psum_f[:], fpt0[:, ts_], wfp0_s[:, ns], start=True, stop=False
                )
                nc.tensor.matmul(
                    psum_f[:], fpt1[:, ts_], wfp1_s[:, ns], start=False, stop=True
                )
                # three short cross-engine chains; psum_i/psum_f are freed at
                # the first hop so the PE's bank rotation never blocks
                zc = dqpool.tile([P, NSZ], f32, tag="zc")
                nc.scalar.activation(
                    zc[:], rwB[:, ns], mybir.ActivationFunctionType.Copy,
                    scale=zro[t][:, 0:1],
                )
                t1 = dqpool.tile([P, NSZ], f32, tag="t1")
                nc.scalar.activation(
                    t1[:], psum_i[:], mybir.ActivationFunctionType.Copy,
                    scale=scl[t][:, 0:1],
                )
                if dve_mult:
                    nc.vector.tensor_tensor(
                        t1[:], t1[:], wsB[:, ns], AluOpType.mult
                    )
                else:
                    nc.gpsimd.tensor_tensor(
                        t1[:], t1[:], wsB[:, ns], AluOpType.mult
                    )
                nc.vector.tensor_tensor(zc[:], zc[:], psum_f[:], AluOpType.add)
                outt = dqpool.tile([P, NSZ], f16, tag="outt")
                nc.vector.tensor_tensor(outt[:], zc[:], t1[:], AluOpType.add)
                nc.scalar.dma_start(out_d[ts_, ns], outt[:])

            # phase A: quantize pipelined with the n=0..2 matmuls so the
            # PE stays fed while the per-tile quantize chains drain; weight
            # chunks prefetched ahead of use
            quantize_tile(0)
            wq0 = load_wq(0)
            wq1 = load_wq(1)
            quantize_tile(1)
            mm_tile(0, 0, wq0, dve_mult=True)
            quantize_tile(2)
            mm_tile(0, 1, wq0, dve_mult=True)
            mm_tile(1, 0, wq1, dve_mult=True)
            wq2 = load_wq(2)
            for t in range(3, TOKT):
                quantize_tile(t)
                mm_tile(0, t - 1, wq0, dve_mult=True)
                mm_tile(1, t - 2, wq1, dve_mult=True)
                mm_tile(2, t - 3, wq2, dve_mult=True)
            mm_tile(0, TOKT - 1, wq0, dve_mult=True)
            for t in range(TOKT - 2, TOKT):
                mm_tile(1, t, wq1, dve_mult=True)
            for t in range(TOKT - 3, TOKT):
                mm_tile(2, t, wq2, dve_mult=True)
            wq_next = load_wq(3)
            for n in range(3, NOUT):
                wq_cur = wq_next
                wq_next = load_wq(n + 1) if n + 1 < NOUT else None
                for t in range(TOKT):
                    mm_tile(n, t, wq_cur)
    _split_multiwait_instructions(nc)
    return nc


def _get_program():
    if "nc" not in _prog_cache:
        _prog_cache["nc"] = _build_program()
    return _prog_cache["nc"]


def make_inputs(x, int_weight, fp_weight, bias, weights_scales, reduced_w,
                int_indices, fp_indices):
    x2 = np.asarray(x, dtype=np.float16).reshape(-1, IN)
    ii = np.asarray(int_indices).astype(np.int64)
    fi = np.asarray(fp_indices).astype(np.int64)

    # fp8 int weights in matmul-ready layout: wq[n,p,k,j] = W[512n+j, 128k+p]
    wint = np.asarray(int_weight).astype(np.float32)  # [OUT, INT]
    wq = np.ascontiguousarray(
        wint.T.reshape(KC, P, NOUT, NSZ).transpose(2, 1, 0, 3)
    ).astype(ml_dtypes.float8_e4m3)

    wfp_all = np.ascontiguousarray(np.asarray(fp_weight, dtype=np.float16).T)
    rw_row = np.ascontiguousarray(np.asarray(reduced_w, dtype=np.float16).reshape(-1))
    wsrow = np.ascontiguousarray(
        np.asarray(weights_scales, dtype=np.float16).reshape(-1)
    )

    xs_all = np.ascontiguousarray(x2[:, ii])
    fpx_all = np.ascontiguousarray(x2[:, fi])

    in_maps = []
    for c in range(N_CORES):
        sl = slice(c * NT, (c + 1) * NT)
        in_maps.append({
            "xs": xs_all[sl],
            "fpx": fpx_all[sl],
            "wq": wq,
            "wfp0": np.ascontiguousarray(wfp_all[0:P]),
            "wfp1": np.ascontiguousarray(wfp_all[P:FP]),
            "wsrow": wsrow,
            "rwrow": rw_row,
        })
    return in_maps


def kernel(x, int_weight, fp_weight, bias, weights_scales, reduced_w,
           int_indices, fp_indices):
    in_maps = make_inputs(x, int_weight, fp_weight, bias, weights_scales,
                          reduced_w, int_indices, fp_indices)
    nc = _get_program()
    res = run_bass_kernel_spmd(nc, in_maps, list(range(N_CORES)))
    out = np.concatenate(
        [res.results[c]["out"] for c in range(N_CORES)], axis=0
    )
    out = out.reshape(B, S, OUT).astype(np.float16)
    b16 = np.asarray(bias, dtype=np.float16).reshape(-1)
    if b16.any():
        # bias is all-zeros per the model spec; slow exact fallback otherwise
        out = (out.astype(np.float32) + b16.astype(np.float32)).astype(np.float16)
    return out


# revision 44
# speedup vs baseline: 1.0107x; 1.0107x over previous
"""MixedQLinear (QUIK-style int4+fp16 outlier linear) on 8 TRN2 NeuronCores.

Sharding: token-parallel. x [4,2048,4096] -> 8192 tokens, 1024 per core;
weights replicated. Each core quantizes its tokens, runs the int4 GEMM in
fp8e4 DoubleRow perf mode (exact: int4 values/products are exactly
representable in e4m3 and accumulate exactly in fp32 PSUM), plus the
fp16 outlier GEMM, dequantizes, and writes its [1024,4096] output slice.

Key algebra: with r = clip(round((x-mn)/scale),0,15)  (no -8 shift),
  out = (sum_k r*Wint) * scale * ws  +  mn*reduced_w  +  fp_x@Wfp^T + bias
(the -8 shift of the reference cancels exactly against zero*reduced_w).
mn (an exact f16 value: the min of f16 inputs) and a ones-row ride as two
extra contraction rows of the fp-outlier matmul with weights [bias; rw].

Quantize trick: r+1024 = x*inv + (1024 - mn*inv) computed on the scalar
engine in one activation; f16 spacing on [1024,2048) is exactly 1.0, so the
f32->f16 output conversion IS round-to-nearest-even onto the integer grid.
The -1024 is subtracted in the PSUM->SBUF fp8 conversion after the PE
transpose (exact integers throughout).

Pipeline: per 128-token tile: load -> min/max -> quantize -> transpose ->
fp8; the tile's n=0 matmuls are emitted immediately after, so the tensor
engine starts ~10us in instead of waiting ~250us for full quantization
(the baseline's big stall).
"""

import numpy as np
import ml_dtypes
import concourse.bass as bass
import concourse.tile as tile
import concourse.mybir as mybir
from concourse.bass_utils import run_bass_kernel_spmd
from concourse.masks import make_identity
from bass_rust import ScopedClock, SyncInfo
from concourse.alu_op_type import AluOpType

# ---------------------------------------------------------------------------
# Workaround: this toolchain's walrus accepts at most one sync-wait on a
# TPB_CTRL (Drain) instruction; Tile's tail drain attaches one wait per
# active DMA queue. Split it into a chain of single-wait drains.
def _drain_and_barrier(self, tick_clock, wait_clock):
    drain_inst = self.nc.sync.drain()
    wait_clock.add_sem_waits(
        drain_inst.ins, ScopedClock({None: tick_clock.global_clock})
    )
    si = drain_inst.ins.sync_info
    ow = list(si.on_wait) if si is not None else []
    if len(ow) > 1:
        si.on_wait = [ow[0]]
        for w in ow[1:]:
            d2 = self.nc.sync.drain()
            d2.ins.sync_info = SyncInfo(on_wait=[w], on_update=[])
    self.nc.all_engine_barrier()
    assert self.sems is not None
    popped = self.nc._tile_sem_poison_stack.pop()
    assert popped is self._sem_poison
    self.nc.clear_and_free_semaphores(list(self.sems.allocated().values()))
    self.nc.all_engine_barrier()


tile.TileContext._drain_and_barrier = _drain_and_barrier


def _split_multiwait_instructions(nc):
    """Walrus here allows only one sync-wait per instruction: hoist extra
    waits onto same-engine NOPs inserted immediately before."""
    ctr = 0
    for fn in nc.m.functions:
        for bb in fn.blocks:
            insts = bb.instructions
            out = []
            changed = False
            for ins in insts:
                si = getattr(ins, "sync_info", None)
                ow = list(si.on_wait) if si is not None else []
                if len(ow) > 1:
                    changed = True
                    for w in ow[:-1]:
                        ctr += 1
                        out.append(
                            mybir.InstNoOp(
                                name=f"mwsplit-{ctr}",
                                sync_info=SyncInfo(on_wait=[w], on_update=[]),
                                engine=ins.engine,
                                bass_nofuse=True,
                            )
                        )
                    si.on_wait = [ow[-1]]
                out.append(ins)
            if changed:
                bb.instructions = out
# ---------------------------------------------------------------------------

N_CORES = 8
B, S, IN, OUT, FP = 4, 2048, 4096, 4096, 256
INT = IN - FP                    # 3840 int-quantized features
NT = (B * S) // N_CORES          # 1024 tokens per core
P = 128
KC = INT // P                    # 30 feature chunks
KH = KC // 2                     # 15 chunks per transpose-staging half
NOUT = 8                         # out-feature chunks
NSZ = OUT // NOUT                # 512
TOKT = NT // P                   # 8 token tiles of 128
BIG = 30000.0

f16 = mybir.dt.float16
f32 = mybir.dt.float32
f8 = mybir.dt.float8e4

_prog_cache = {}


def _build_program():
    nc = bass.Bass()
    xs = nc.declare_dram_parameter("xs", [NT, INT], f16, isOutput=False)
    fpx = nc.declare_dram_parameter("fpx", [NT, FP], f16, isOutput=False)
    wq = nc.declare_dram_parameter("wq", [NOUT, P, KC, NSZ], f8, isOutput=False)
    wfp0 = nc.declare_dram_parameter("wfp0", [P, OUT], f16, isOutput=False)
    wfp1 = nc.declare_dram_parameter("wfp1", [P, OUT], f16, isOutput=False)
    wsrow = nc.declare_dram_parameter("wsrow", [OUT], f16, isOutput=False)
    rwrow = nc.declare_dram_parameter("rwrow", [OUT], f16, isOutput=False)
    out_d = nc.declare_dram_parameter("out", [NT, OUT], f16, isOutput=True)

    def bcast(ap, parts=P):
        # DRAM row -> all partitions: stride-0 partition dim, SWDGE DMA
        return bass.AP(
            tensor=ap.tensor, offset=ap.offset, ap=[[0, parts]] + list(ap.ap)
        )

    HSZ = NT // 2
    DR = mybir.MatmulPerfMode.DoubleRow

    with tile.TileContext(nc) as tc:
        with (
            tc.tile_pool(name="const", bufs=1) as cpool,
            tc.tile_pool(name="wqp", bufs=3) as wqpool,
            tc.tile_pool(name="xp", bufs=1) as xpool,
            tc.tile_pool(name="r16p", bufs=1) as r16pool,
            tc.tile_pool(name="rtp", bufs=1) as rtpool,
            tc.tile_pool(name="s1", bufs=1) as s1pool,
            tc.tile_pool(name="scr", bufs=1) as scrpool,
            tc.tile_pool(name="dq", bufs=2) as dqpool,
            tc.tile_pool(name="pi", bufs=2, space="PSUM") as pipool,
            tc.tile_pool(name="pf", bufs=2, space="PSUM") as pfpool,
            tc.tile_pool(name="st", bufs=2, space="PSUM") as stpool,
            tc.tile_pool(name="dram", bufs=1, space="DRAM") as dpool,
        ):
            # ---- resident constants -------------------------------------
            # identity is built first: every PE transpose waits on it, and the
            # gpsimd queue is in-order
            ident = cpool.tile([P, P], f16, tag="ident")
            make_identity(nc, ident[:])

            # fp-outlier activations: natural (fast) DMA first on the sync
            # ring, then transposed on the PE -- the scheduler puts the (0,0)
            # fp matmuls at the head of the in-order PE queue, so fpt must be
            # ready early, and transposed DMAs are far too slow for that
            # x(0), x(1) lead the sync ring (they gate the quantize ramp),
            # then fpx (needed by ~25us), then the remaining x tiles;
            # weights ride the scalar ring so neither blocks the other
            xts = [None] * TOKT
            for t in range(2):
                ts_ = slice(t * P, (t + 1) * P)
                xt = xpool.tile([P, INT], f16, name=f"xt{t}", tag=f"xt{t}")
                nc.sync.dma_start(xt[:], xs[ts_, :])
                xts[t] = xt
            fxts = []
            for t in range(TOKT):
                ts_ = slice(t * P, (t + 1) * P)
                fxt = cpool.tile([P, FP], f16, name=f"fxt{t}", tag=f"fxt{t}")
                nc.sync.dma_start(fxt[:], fpx[ts_, :])
                fxts.append(fxt)
            for t in range(2, TOKT):
                ts_ = slice(t * P, (t + 1) * P)
                # last two tiles reuse the first two buffers (WAR-sequenced
                # behind their consumption, still ahead of the pipeline)
                xt = xpool.tile([P, INT], f16, name=f"xt{t}", tag=f"xt{t % 6}")
                nc.sync.dma_start(xt[:], xs[ts_, :])
                xts[t] = xt
            # ws/rw rows first on the scalar ring (tiny, needed early by the
            # PE outer-product broadcasts), then fp weights, then wq chunks
            wsrow_s = cpool.tile([1, OUT], f16, tag="wsrow_s")
            nc.scalar.dma_start(wsrow_s[:], wsrow[:].rearrange("(a j) -> a j", a=1))
            rwrow_s = cpool.tile([1, OUT], f16, tag="rwrow_s")
            nc.scalar.dma_start(rwrow_s[:], rwrow[:].rearrange("(a j) -> a j", a=1))
            wfp0_s = cpool.tile([P, OUT], f16, tag="wfp0")
            nc.scalar.dma_start(wfp0_s[:], wfp0[:])
            wfp1_s = cpool.tile([P, OUT], f16, tag="wfp1")
            nc.scalar.dma_start(wfp1_s[:], wfp1[:])
            # wsB/rwB = ones x row outer products on the PE (SWDGE bcast is slow)
            ones1 = cpool.tile([1, P], f16, tag="ones1")
            nc.vector.memset(ones1[:], 1.0)
            wsB = cpool.tile([P, OUT], f16, tag="wsB")
            rwB = cpool.tile([P, OUT], f16, tag="rwB")
            for n in range(NOUT):
                ns = slice(n * NSZ, (n + 1) * NSZ)
                pws = pfpool.tile([P, NSZ], f32, name="pws", tag="pf")
                nc.tensor.matmul(pws[:], ones1[:], wsrow_s[:, ns],
                                 start=True, stop=True)
                nc.scalar.copy(wsB[:, ns], pws[:])
                prw = pfpool.tile([P, NSZ], f32, name="prw", tag="pf")
                nc.tensor.matmul(prw[:], ones1[:], rwrow_s[:, ns],
                                 start=True, stop=True)
                nc.scalar.copy(rwB[:, ns], prw[:])
            # PE-transpose fpx tiles into [feat, tok] layout
            fpt0 = cpool.tile([P, NT], f16, tag="fpt0")
            fpt1 = cpool.tile([P, NT], f16, tag="fpt1")
            for t in range(TOKT):
                ts_ = slice(t * P, (t + 1) * P)
                stgf = stpool.tile([P, KH, P], f16, name="stgf", tag="stg")
                nc.tensor.transpose(stgf[:, 0, :], fxts[t][:, 0:P], ident[:])
                nc.tensor.transpose(stgf[:, 1, :], fxts[t][:, P:FP], ident[:])
                nc.scalar.copy(fpt0[:, ts_], stgf[:, 0, :])
                nc.scalar.copy(fpt1[:, ts_], stgf[:, 1, :])

            def load_wq(n):
                w = wqpool.tile([P, KC, NSZ], f8, tag="wq")
                nc.sync.dma_start(w[:], wq[n])
                return w

            scl = [None] * TOKT   # per-tile [128,1] f32 scale (dequant)
            zro = [None] * TOKT   # per-tile [128,1] f32 zero point
            rts = [None] * TOKT   # per-tile [128, KC, 128] fp8 r values

            def quantize_tile(t):
                ts_ = slice(t * P, (t + 1) * P)
                xt = xts[t]
                mx_t = s1pool.tile([P, 1], f32, tag=f"mx{t}")
                mn_t = s1pool.tile([P, 1], f32, tag=f"mn{t}")
                scr = scrpool.tile([P, INT // 2], f16, tag="scr")
                nc.vector.tensor_tensor(
                    scr[:], xt[:, : INT // 2], xt[:, INT // 2 :], AluOpType.max
                )
                nc.vector.tensor_reduce(
                    mx_t[:], scr[:], mybir.AxisListType.X, AluOpType.max
                )
                scr2 = scrpool.tile([P, INT // 2], f16, tag="scr")
                nc.vector.tensor_tensor(
                    scr2[:], xt[:, : INT // 2], xt[:, INT // 2 :], AluOpType.min
                )
                nc.vector.tensor_reduce(
                    mn_t[:], scr2[:], mybir.AxisListType.X, AluOpType.min
                )
                sc_t = s1pool.tile([P, 1], f32, tag=f"sc{t}")
                nc.vector.tensor_tensor(sc_t[:], mx_t[:], mn_t[:], AluOpType.subtract)
                nc.vector.tensor_scalar(
                    sc_t[:], sc_t[:], 1.0 / 15.0, 1e-8,
                    AluOpType.mult, AluOpType.max,
                )
                inv_t = s1pool.tile([P, 1], f32, tag=f"inv{t}")
                nc.vector.reciprocal(inv_t[:], sc_t[:])
                # nmi = 1024 - mn*inv (quantize bias)
                nmi = s1pool.tile([P, 1], f32, tag=f"nmi{t}")
                nc.vector.tensor_tensor(nmi[:], mn_t[:], inv_t[:], AluOpType.mult)
                nc.vector.tensor_scalar(
                    nmi[:], nmi[:], -1.0, 1024.0, AluOpType.mult, AluOpType.add
                )
                # zero point zero = 8*scale + mn (stays f32 for dequant)
                zr = s1pool.tile([P, 1], f32, tag=f"zr{t}")
                nc.vector.tensor_scalar(
                    zr[:], sc_t[:], 8.0, mn_t[:, 0:1], AluOpType.mult, AluOpType.add
                )
                # r+1024 in one Pool op; f16 write rounds to the int grid
                r16 = r16pool.tile([P, INT], f16, tag="r16")
                nc.gpsimd.tensor_scalar(
                    r16[:], xt[:], inv_t[:, 0:1], nmi[:, 0:1],
                    AluOpType.mult, AluOpType.add,
                )
                rt = rtpool.tile([P, KC, P], f8, tag=f"rt{t}")
                for h in range(2):
                    stg = stpool.tile([P, KH, P], f16, tag="stg")
                    for kk in range(KH):
                        c = h * KH + kk
                        nc.tensor.transpose(
                            stg[:, kk, :], r16[:, c * P : (c + 1) * P], ident[:]
                        )
                    nc.scalar.activation(
                        rt[:, h * KH : (h + 1) * KH, :], stg[:],
                        mybir.ActivationFunctionType.Copy, bias=-1032.0,
                    )
                scl[t] = sc_t
                zro[t] = zr
                rts[t] = rt

            def mm_tile(n, t, w, dve_mult=False):
                ts_ = slice(t * P, (t + 1) * P)
                ns = slice(n * NSZ, (n + 1) * NSZ)
                psum_i = pipool.tile([P, NSZ], f32, tag="pi")
                for kp in range(KC // 2):
                    nc.tensor.matmul(
                        psum_i[:],
                        rts[t][:, 2 * kp : 2 * kp + 2, :],
                        w[:, 2 * kp : 2 * kp + 2, :],
                        start=(kp == 0), stop=(kp == KC // 2 - 1),
                        perf_mode=DR,
                    )
                psum_f = pfpool.tile([P, NSZ], f32, tag="pf")
                nc.tensor.matmul(
                    psum_f[:], fpt0[:, ts_], wfp0_s[:, ns], start=True, stop=False
                )
                nc.tensor.matmul(
                    psum_f[:], fpt1[:, ts_], wfp1_s[:, ns], start=False, stop=True
                )
                # three short cross-engine chains; psum_i/psum_f are freed at
                # the first hop so the PE's bank rotation never blocks
                zc = dqpool.tile([P, NSZ], f32, tag="zc")
                nc.scalar.activation(
                    zc[:], rwB[:, ns], mybir.ActivationFunctionType.Copy,
                    scale=zro[t][:, 0:1],
                )
                t1 = dqpool.tile([P, NSZ], f32, tag="t1")
                nc.scalar.activation(
                    t1[:], psum_i[:], mybir.ActivationFunctionType.Copy,
                    scale=scl[t][:, 0:1],
                )
                if dve_mult:
                    nc.vector.tensor_tensor(
                        t1[:], t1[:], wsB[:, ns], AluOpType.mult
                    )
                else:
                    nc.gpsimd.tensor_tensor(
                        t1[:], t1[:], wsB[:, ns], AluOpType.mult
                    )
                nc.vector.tensor_tensor(zc[:], zc[:], psum_f[:], AluOpType.add)
                outt = dqpool.tile([P, NSZ], f16, tag="outt")
                nc.vector.tensor_tensor(outt[:], zc[:], t1[:], AluOpType.add)
                nc.scalar.dma_start(out_d[ts_, ns], outt[:])

            # phase A: quantize pipelined with the n=0..2 matmuls so the
            # PE stays fed while the per-tile quantize chains drain; weight
            # chunks prefetched ahead of use
            quantize_tile(0)
            wq0 = load_wq(0)
            wq1 = load_wq(1)
            quantize_tile(1)
            mm_tile(0, 0, wq0, dve_mult=True)
            quantize_tile(2)
            mm_tile(0, 1, wq0, dve_mult=True)
            mm_tile(1, 0, wq1, dve_mult=True)
            wq2 = load_wq(2)
            for t in range(3, TOKT):
                quantize_tile(t)
                mm_tile(0, t - 1, wq0, dve_mult=True)
                mm_tile(1, t - 2, wq1, dve_mult=True)
                mm_tile(2, t - 3, wq2, dve_mult=True)
            mm_tile(0, TOKT - 1, wq0, dve_mult=True)
            for t in range(TOKT - 2, TOKT):
                mm_tile(1, t, wq1, dve_mult=True)
            for t in range(TOKT - 3, TOKT):
                mm_tile(2, t, wq2, dve_mult=True)
            wq_next = load_wq(3)
            for n in range(3, NOUT):
                wq_cur = wq_next
                wq_next = load_wq(n + 1) if n + 1 < NOUT else None
                for t in range(TOKT):
                    # last blocks skip the Pool hop so the drain tail is short
                    mm_tile(n, t, wq_cur,
                            dve_mult=(n == NOUT - 1 and t >= TOKT - 2))
    _split_multiwait_instructions(nc)
    return nc


def _get_program():
    if "nc" not in _prog_cache:
        _prog_cache["nc"] = _build_program()
    return _prog_cache["nc"]


def make_inputs(x, int_weight, fp_weight, bias, weights_scales, reduced_w,
                int_indices, fp_indices):
    x2 = np.asarray(x, dtype=np.float16).reshape(-1, IN)
    ii = np.asarray(int_indices).astype(np.int64)
    fi = np.asarray(fp_indices).astype(np.int64)

    # fp8 int weights in matmul-ready layout: wq[n,p,k,j] = W[512n+j, 128k+p]
    wint = np.asarray(int_weight).astype(np.float32)  # [OUT, INT]
    wq = np.ascontiguousarray(
        wint.T.reshape(KC, P, NOUT, NSZ).transpose(2, 1, 0, 3)
    ).astype(ml_dtypes.float8_e4m3)

    wfp_all = np.ascontiguousarray(np.asarray(fp_weight, dtype=np.float16).T)
    rw_row = np.ascontiguousarray(np.asarray(reduced_w, dtype=np.float16).reshape(-1))
    wsrow = np.ascontiguousarray(
        np.asarray(weights_scales, dtype=np.float16).reshape(-1)
    )

    xs_all = np.ascontiguousarray(x2[:, ii])
    fpx_all = np.ascontiguousarray(x2[:, fi])

    in_maps = []
    for c in range(N_CORES):
        sl = slice(c * NT, (c + 1) * NT)
        in_maps.append({
            "xs": xs_all[sl],
            "fpx": fpx_all[sl],
            "wq": wq,
            "wfp0": np.ascontiguousarray(wfp_all[0:P]),
            "wfp1": np.ascontiguousarray(wfp_all[P:FP]),
            "wsrow": wsrow,
            "rwrow": rw_row,
        })
    return in_maps


def kernel(x, int_weight, fp_weight, bias, weights_scales, reduced_w,
           int_indices, fp_indices):
    in_maps = make_inputs(x, int_weight, fp_weight, bias, weights_scales,
                          reduced_w, int_indices, fp_indices)
    nc = _get_program()
    res = run_bass_kernel_spmd(nc, in_maps, list(range(N_CORES)))
    out = np.concatenate(
        [res.results[c]["out"] for c in range(N_CORES)], axis=0
    )
    out = out.reshape(B, S, OUT).astype(np.float16)
    b16 = np.asarray(bias, dtype=np.float16).reshape(-1)
    if b16.any():
        # bias is all-zeros per the model spec; slow exact fallback otherwise
        out = (out.astype(np.float32) + b16.astype(np.float32)).astype(np.float16)
    return out
